# revision 1
# baseline (speedup 1.0000x reference)
"""nn_CrossAttention kernel for 8x TRN2 NeuronCores.

Sharding: core c = (batch b = c//2, head-group hg = c%2 of 8 heads).
Each core: projections (f32r matmuls), scoresT = K^T-layout QK^T with
2-head PE row-packing, exp on ACT (scale 1/8 fused), A*V with a
ones-augmented V (extra output row = softmax denominator), normalize via
K=1 broadcast matmul + DVE multiply. Host pre-transposes activations and
splits W columns per head-group; host re-assembles the [512,2048] per-core
ctxT outputs into the full [4,2048,1024] output.
"""

import json
import numpy as np

B, S, D, NH, HD = 4, 2048, 1024, 16, 64
CPC = 512          # cols per core = 8 heads * 64
NCORES = 8
NDT = D // 128     # 8 d-tiles
NP = CPC // 128    # 4 c-tiles (head pairs)
NSK = S // 128     # 16 sk-tiles
NJ = S // 512      # 4 sq chunks
SQC = 512          # sq chunk size


# ---------------------------------------------------------------- drain fix
def _fix_module_json(bj: bytes) -> bytes:
    """This walrus build accepts at most ONE sync wait/update on CTRL-lowered
    instructions (Drain). Move extras onto EventSemaphore instructions."""
    d = json.loads(bj)
    counter = [0]

    def fix_block(b):
        out = []
        for inst in b.get("instructions", []):
            si = inst.get("sync_info") or {}
            ow = si.get("on_wait") or []
            ou = si.get("on_update") or []
            if (inst.get("opcode") not in
                    ("EventSemaphore", "Call", "RegisterMove",
                     "UnconditionalBranch", "ISA", "Drain") and len(ow) > 1):
                # Several instruction structs in this walrus have room for
                # only one sync wait; hoist extras onto EventSemaphores
                # issued just before on the same engine (engine streams are
                # serial, so the blocking point is identical). Updates stay.
                for w in ow[1:]:
                    counter[0] += 1
                    out.append({
                        "debug": inst.get("debug", 0),
                        "engine": inst["engine"],
                        "ins": [], "outs": [],
                        "name": f"synthmmw-{counter[0]}",
                        "opcode": "EventSemaphore",
                        "sync_info": {"on_update": [], "on_wait": [w]},
                    })
                inst["sync_info"] = {"on_update": ou, "on_wait": ow[:1]}
                out.append(inst)
                continue
            if inst.get("opcode") == "Drain" and (len(ow) > 1 or len(ou) > 1):
                for w in ow[1:]:
                    counter[0] += 1
                    out.append({
                        "debug": inst.get("debug", 0),
                        "engine": inst["engine"],
                        "ins": [], "outs": [],
                        "name": f"synthwait-{counter[0]}",
                        "opcode": "EventSemaphore",
                        "sync_info": {"on_update": [], "on_wait": [w]},
                    })
                inst["sync_info"] = {"on_update": ou[:1], "on_wait": ow[:1]}
                out.append(inst)
                for u in ou[1:]:
                    counter[0] += 1
                    out.append({
                        "debug": inst.get("debug", 0),
                        "engine": inst["engine"],
                        "ins": [], "outs": [],
                        "name": f"synthupd-{counter[0]}",
                        "opcode": "EventSemaphore",
                        "sync_info": {"on_update": [u], "on_wait": []},
                    })
            else:
                out.append(inst)
        b["instructions"] = out
        for sb in b.get("blocks", []):
            fix_block(sb)

    for fn in d.get("functions", []):
        for blk in fn.get("blocks", []):
            fix_block(blk)
    return json.dumps(d).encode()


def _install_drainfix():
    import concourse.bass as bass
    if getattr(bass.Bass, "_drainfix_installed", False):
        return
    orig = bass.Bass.to_json_bytes

    def patched(self):
        return _fix_module_json(orig(self))

    bass.Bass.to_json_bytes = patched
    bass.Bass._drainfix_installed = True


# ---------------------------------------------------------------- program
_NC_CACHE = []


def _build_nc(reps=1):
    import concourse.bass as bass
    import concourse.mybir as mybir
    from concourse.tile import TileContext
    from contextlib import ExitStack

    f32 = mybir.dt.float32
    f32r = mybir.dt.float32r
    bf16 = mybir.dt.bfloat16
    EXP = mybir.ActivationFunctionType.Exp

    nc = bass.Bass("TRN2", num_devices=NCORES)

    xqT = nc.dram_tensor("xqT", [D, S], f32, kind="ExternalInput")
    xkT = nc.dram_tensor("xkT", [D, S], f32, kind="ExternalInput")
    xvT = nc.dram_tensor("xvT", [D, S], f32, kind="ExternalInput")
    wq = nc.dram_tensor("wq", [D, CPC], f32, kind="ExternalInput")
    wk = nc.dram_tensor("wk", [D, CPC], f32, kind="ExternalInput")
    wv = nc.dram_tensor("wv", [D, CPC], f32, kind="ExternalInput")
    bqd = nc.dram_tensor("bq", [CPC], f32, kind="ExternalInput")
    bkd = nc.dram_tensor("bk", [CPC], f32, kind="ExternalInput")
    bvd = nc.dram_tensor("bv", [CPC], f32, kind="ExternalInput")
    conesd = nc.dram_tensor("cones", [1, 64], f32, kind="ExternalInput")
    outd = nc.dram_tensor("out", [CPC, S], f32, kind="ExternalOutput")

    with ExitStack() as ctx:
        ctx.enter_context(nc.allow_low_precision(
            reason="f32r tiles are full fp32 storage; matmul accumulates f32"))
        tc = ctx.enter_context(TileContext(nc))
        sb = ctx.enter_context(tc.tile_pool(name="sb", bufs=1))
        ps = ctx.enter_context(tc.tile_pool(name="ps", bufs=1, space="PSUM"))

        # ---- constants ----
        bq_sb = sb.tile([128, NP], f32, name="bq_sb")
        nc.sync.dma_start(out=bq_sb, in_=bqd.rearrange("(p c) -> c p", p=NP))
        bk_sb = sb.tile([128, NP], f32, name="bk_sb")
        nc.sync.dma_start(out=bk_sb, in_=bkd.rearrange("(p c) -> c p", p=NP))
        bv_bc = sb.tile([128, CPC], f32, name="bv_bc")
        _bva = bvd[:]
        nc.sync.dma_start(
            out=bv_bc,
            in_=bass.AP(tensor=_bva.tensor, offset=_bva.offset,
                        ap=[[0, 128]] + list(_bva.ap)))
        ones = sb.tile([1, 64], f32r, name="ones")
        nc.sync.dma_start(out=ones, in_=conesd[:, :].bitcast(f32r))

        # wv resident [128, dd, 512]
        wv_sb = sb.tile([128, NDT, CPC], f32r, name="wv_sb")
        for dd in range(NDT):
            nc.sync.dma_start(out=wv_sb[:, dd, :], in_=wv[dd * 128:(dd + 1) * 128, :].bitcast(f32r))

        # resident qT/kT [c 128, s 2048] per head pair
        qT = [sb.tile([128, S], f32r, name=f"qT{p}") for p in range(NP)]
        kT = [sb.tile([128, S], f32r, name=f"kT{p}") for p in range(NP)]
        # V augmented with ones column, bf16: [sk_in_tile, sk_tile, head, 65]
        v_aug = sb.tile([128, NSK, 8, 65], bf16, name="v_aug")
        nc.gpsimd.memset(v_aug[:, :, :, 64:65], 1.0)

        # ---- helpers ----
        def emit_proj_qk(t, plist, w_dram, x_dram, bias_sb, dst):
            """dst[p][:, t*512:(t+1)*512] = (x @ W + b).T chunk; contract D."""
            xts, wts = [], []
            for dd in range(NDT):
                x_t = sb.tile([128, SQC], f32r, tag="xs", bufs=10, name=f"x_{t}_{dd}")
                nc.sync.dma_start(
                    out=x_t, in_=x_dram[dd * 128:(dd + 1) * 128, t * SQC:(t + 1) * SQC].bitcast(f32r))
                xts.append(x_t)
                c0, c1 = plist[0] * 128, (plist[-1] + 1) * 128
                w_t = sb.tile([128, c1 - c0], f32r, tag="ws", bufs=10, name=f"w_{t}_{dd}")
                nc.sync.dma_start(out=w_t, in_=w_dram[dd * 128:(dd + 1) * 128, c0:c1].bitcast(f32r))
                wts.append(w_t)
            for p in plist:
                pr = ps.tile([128, SQC], f32, tag="vp", bufs=2, name=f"prj_{t}_{p}")
                off = (p - plist[0]) * 128
                for dd in range(NDT):
                    nc.tensor.matmul(
                        pr[:, :],
                        wts[dd][:, off:off + 128],
                        xts[dd][:, :],
                        start=(dd == 0), stop=(dd == NDT - 1))
                nc.vector.tensor_scalar_add(
                    dst[p][:, t * SQC:(t + 1) * SQC], pr[:, :], bias_sb[:, p:p + 1])

        def emit_proj_v(tt_list):
            """v_aug[:, tt, h, 0:64] = (xv @ Wv + bv) rows tt*128.., bf16."""
            for tt in tt_list:
                xvt = []
                for dd in range(NDT):
                    xv_t = sb.tile([128, 128], f32r, tag="xv", bufs=6, name=f"xv_{tt}_{dd}")
                    nc.sync.dma_start(
                        out=xv_t,
                        in_=xvT[dd * 128:(dd + 1) * 128, tt * 128:(tt + 1) * 128].bitcast(f32r))
                    xvt.append(xv_t)
                pv = ps.tile([128, CPC], f32, tag="vp", bufs=2, name=f"pv_{tt}")
                for dd in range(NDT):
                    nc.tensor.matmul(
                        pv[:, :],
                        xvt[dd][:, :],
                        wv_sb[:, dd, :],
                        start=(dd == 0), stop=(dd == NDT - 1))
                nc.vector.tensor_add(
                    v_aug[:, tt, :, 0:64],
                    pv.rearrange("c (h d) -> c h d", h=8),
                    bv_bc.rearrange("c (h d) -> c h d", h=8))

        alpha = {}  # (p, j, h, g) -> tile [128, 2, 512] bf16 (sk pair g)

        def emit_qkexp(p, j):
            """scoresT + exp for pair p, sq chunk j. sk-tiles in pairs g."""
            for g in range(NSK // 2):
                sc = []
                for h in range(2):
                    s_h = ps.tile([128, 2, SQC], f32, tag="scores", bufs=2,
                                  name=f"sc_{p}_{j}_{g}_{h}")
                    sc.append(s_h)
                for u in range(2):  # sk-tile i = 2g+u
                    i = 2 * g + u
                    for h in range(2):
                        nc.tensor.matmul(
                            sc[h][:, u, :],
                            kT[p][h * 64:(h + 1) * 64, i * 128:(i + 1) * 128],
                            qT[p][h * 64:(h + 1) * 64, j * SQC:(j + 1) * SQC],
                            start=True, stop=True)
                for h in range(2):
                    a_t = sb.tile([128, 2, SQC], bf16, tag="alpha", bufs=16,
                                  name=f"al_{p}_{j}_{g}_{h}")
                    nc.scalar.activation(a_t[:, :, :], sc[h][:, :, :], EXP, scale=0.125)
                    alpha[(p, j, h, g)] = a_t

        def emit_av(p, j):
            """ctxT rows for pair p chunk j: accumulate over sk, normalize, out."""
            for h in range(2):
                av = ps.tile([65, SQC], f32, tag="av", bufs=2, name=f"av_{p}_{j}_{h}")
                for g in range(NSK // 2):
                    a_t = alpha.pop((p, j, h, g))
                    for u in range(2):
                        i = 2 * g + u
                        nc.tensor.matmul(
                            av[:, :],
                            v_aug[:, i, 2 * p + h, :],
                            a_t[:, u, :],
                            start=(i == 0), stop=(i == NSK - 1))
                rec = sb.tile([1, SQC], f32r, tag="rec", bufs=4, name=f"rec_{p}_{j}_{h}")
                nc.vector.reciprocal(rec[:, :], av[64:65, :])
                bc = ps.tile([64, SQC], f32, tag="vp", bufs=2, name=f"bc_{p}_{j}_{h}")
                nc.tensor.matmul(bc[:, :], ones[:, :],
                                 rec[:, :], start=True, stop=True)
                bcs = sb.tile([64, SQC], f32, tag="bcs", bufs=4, name=f"bcs_{p}_{j}_{h}")
                nc.vector.tensor_copy(bcs[:, :], bc[:, :])
                cx = sb.tile([64, SQC], f32, tag="cx", bufs=4, name=f"cx_{p}_{j}_{h}")
                nc.vector.tensor_mul(cx[:, :], av[0:64, :], bcs[:, :])
                r0 = (2 * p + h) * 64
                nc.sync.dma_start(
                    out=outd[r0:r0 + 64, j * SQC:(j + 1) * SQC], in_=cx[:, :])

        # ---- emission schedule ----
        def _emit_all():
            for t in range(NJ):
                emit_proj_qk(t, [0], wk, xkT, bk_sb, kT)
            for t in range(NJ):
                emit_proj_qk(t, [0], wq, xqT, bq_sb, qT)
            emit_qkexp(0, 0)
            emit_proj_v(range(NSK))
            emit_qkexp(0, 1)
            emit_av(0, 0)
            for t in range(NJ):
                emit_proj_qk(t, [1, 2, 3], wk, xkT, bk_sb, kT)
            emit_qkexp(0, 2)
            emit_av(0, 1)
            for t in range(NJ):
                emit_proj_qk(t, [1, 2, 3], wq, xqT, bq_sb, qT)
            emit_qkexp(0, 3)
            emit_av(0, 2)
            seq = [(p, j) for p in range(NP) for j in range(NJ)]
            prev = [(0, 3)]
            for (p, j) in seq[4:]:
                emit_qkexp(p, j)
                emit_av(*prev.pop(0))
                prev.append((p, j))
            for pj in prev:
                emit_av(*pj)

        for _rep in range(reps):
            _emit_all()

    return nc


_NC_BY_REPS = {}


def _get_nc(reps=1):
    if reps not in _NC_BY_REPS:
        _install_drainfix()
        _NC_BY_REPS[reps] = _build_nc(reps)
    return _NC_BY_REPS[reps]


# ---------------------------------------------------------------- entry
def kernel(query, key_in, value, Wq, bq, Wk, bk, Wv, bv):
    from concourse.bass_utils import run_bass_kernel_spmd

    nc = _get_nc()
    query = np.asarray(query, np.float32)
    key_in = np.asarray(key_in, np.float32)
    value = np.asarray(value, np.float32)
    Wq = np.asarray(Wq, np.float32)
    Wk = np.asarray(Wk, np.float32)
    Wv = np.asarray(Wv, np.float32)
    bq = np.asarray(bq, np.float32)
    bk = np.asarray(bk, np.float32)
    bv = np.asarray(bv, np.float32)

    in_maps = []
    for c in range(NCORES):
        b, hg = divmod(c, 2)
        cols = slice(hg * CPC, (hg + 1) * CPC)
        in_maps.append({
            "xqT": np.ascontiguousarray(query[b].T),
            "xkT": np.ascontiguousarray(key_in[b].T),
            "xvT": np.ascontiguousarray(value[b].T),
            "wq": np.ascontiguousarray(Wq[:, cols]),
            "wk": np.ascontiguousarray(Wk[:, cols]),
            "wv": np.ascontiguousarray(Wv[:, cols]),
            "bq": np.ascontiguousarray(bq[cols]),
            "bk": np.ascontiguousarray(bk[cols]),
            "bv": np.ascontiguousarray(bv[cols]),
            "cones": np.ones((1, 64), np.float32),
        })

    res = run_bass_kernel_spmd(nc, in_maps, core_ids=list(range(NCORES)))

    out = np.empty((B, S, D), np.float32)
    for c in range(NCORES):
        b, hg = divmod(c, 2)
        out[b, :, hg * CPC:(hg + 1) * CPC] = res.results[c]["out"].T
    return out



# revision 16
# speedup vs baseline: 1.1735x; 1.1735x over previous
"""nn_CrossAttention kernel for 8x TRN2 NeuronCores.

Sharding: core c = (batch b = c//2, head-group hg = c%2 of 8 heads).
All-bf16 device pipeline: host packs x = [qT;kT;vT] [3,1024,2048] bf16 and
w = [Wq;Wk;Wv] column-slices [3,1024,512] bf16. Per core: projections
(bf16 matmuls, K=128, N=512), scoresT = K^T-layout QK^T with 2-head PE
row-packing, exp on ACT (scale 1/8 fused, 1024-free instructions,
PSUM->SBUF bf16), A*V with ones-augmented V (extra row = softmax
denominator), normalize via K=1 broadcast matmul + DVE multiply.
Output bf16 [512,2048] per core; host upcasts/reassembles.

PSUM budget: sc ring bufs=3 (6 banks) + proj/bc 1 + av 1 = 8 banks, so the
PE keeps ~1.5 score groups in flight while ACT drains exp.
"""

import json
import numpy as np

B, S, D, NH, HD = 4, 2048, 1024, 16, 64
CPC = 512          # cols per core = 8 heads * 64
NCORES = 8
NDT = D // 128     # 8 d-tiles
NP = CPC // 128    # 4 c-tiles (head pairs)
NSK = S // 128     # 16 sk-tiles
NJ = S // 512      # 4 sq chunks
SQC = 512          # sq chunk size


# ---------------------------------------------------------------- drain fix
def _fix_module_json(bj: bytes) -> bytes:
    """This walrus build accepts at most ONE sync wait/update on CTRL-lowered
    instructions (Drain). Move extras onto EventSemaphore instructions."""
    d = json.loads(bj)
    counter = [0]

    def fix_block(b):
        out = []
        for inst in b.get("instructions", []):
            si = inst.get("sync_info") or {}
            ow = si.get("on_wait") or []
            ou = si.get("on_update") or []
            if (inst.get("opcode") not in
                    ("EventSemaphore", "Call", "RegisterMove",
                     "UnconditionalBranch", "ISA", "Drain") and len(ow) > 1):
                for w in ow[1:]:
                    counter[0] += 1
                    out.append({
                        "debug": inst.get("debug", 0),
                        "engine": inst["engine"],
                        "ins": [], "outs": [],
                        "name": f"synthmmw-{counter[0]}",
                        "opcode": "EventSemaphore",
                        "sync_info": {"on_update": [], "on_wait": [w]},
                    })
                inst["sync_info"] = {"on_update": ou, "on_wait": ow[:1]}
                out.append(inst)
                continue
            if inst.get("opcode") == "Drain" and (len(ow) > 1 or len(ou) > 1):
                for w in ow[1:]:
                    counter[0] += 1
                    out.append({
                        "debug": inst.get("debug", 0),
                        "engine": inst["engine"],
                        "ins": [], "outs": [],
                        "name": f"synthwait-{counter[0]}",
                        "opcode": "EventSemaphore",
                        "sync_info": {"on_update": [], "on_wait": [w]},
                    })
                inst["sync_info"] = {"on_update": ou[:1], "on_wait": ow[:1]}
                out.append(inst)
                for u in ou[1:]:
                    counter[0] += 1
                    out.append({
                        "debug": inst.get("debug", 0),
                        "engine": inst["engine"],
                        "ins": [], "outs": [],
                        "name": f"synthupd-{counter[0]}",
                        "opcode": "EventSemaphore",
                        "sync_info": {"on_update": [u], "on_wait": []},
                    })
            else:
                out.append(inst)
        b["instructions"] = out
        for sb in b.get("blocks", []):
            fix_block(sb)

    for fn in d.get("functions", []):
        for blk in fn.get("blocks", []):
            fix_block(blk)
    return json.dumps(d).encode()


def _install_drainfix():
    import concourse.bass as bass
    if getattr(bass.Bass, "_drainfix_installed", False):
        return
    orig = bass.Bass.to_json_bytes

    def patched(self):
        return _fix_module_json(orig(self))

    bass.Bass.to_json_bytes = patched
    bass.Bass._drainfix_installed = True


# ---------------------------------------------------------------- program
def _build_nc(reps=1):
    import concourse.bass as bass
    import concourse.mybir as mybir
    from concourse.tile import TileContext
    from contextlib import ExitStack

    f32 = mybir.dt.float32
    f32r = mybir.dt.float32r
    bf16 = mybir.dt.bfloat16
    EXP = mybir.ActivationFunctionType.Exp

    nc = bass.Bass("TRN2", num_devices=NCORES)

    xd = nc.dram_tensor("x", [3, D, S], bf16, kind="ExternalInput")
    wd = nc.dram_tensor("w", [3, D, CPC], bf16, kind="ExternalInput")
    biasd = nc.dram_tensor("bias", [3, CPC], f32, kind="ExternalInput")
    outd = nc.dram_tensor("out", [CPC, S], bf16, kind="ExternalOutput")

    with ExitStack() as ctx:
        ctx.enter_context(nc.allow_low_precision(
            reason="bf16 pipeline validated against f32 reference (<1% rel)"))
        tc = ctx.enter_context(TileContext(nc))
        sb = ctx.enter_context(tc.tile_pool(name="sb", bufs=1))
        ps = ctx.enter_context(tc.tile_pool(name="ps", bufs=1, space="PSUM"))

        # ---- constants (once) ----
        bq_sb = sb.tile([128, NP], f32, name="bq_sb")
        nc.sync.dma_start(out=bq_sb, in_=biasd[0].rearrange("(p c) -> c p", p=NP))
        bk_sb = sb.tile([128, NP], f32, name="bk_sb")
        nc.sync.dma_start(out=bk_sb, in_=biasd[1].rearrange("(p c) -> c p", p=NP))
        bv_bc = sb.tile([128, CPC], f32, name="bv_bc")
        _bva = biasd[2]
        nc.sync.dma_start(
            out=bv_bc,
            in_=bass.AP(tensor=_bva.tensor, offset=_bva.offset,
                        ap=[[0, 128]] + list(_bva.ap)))
        ones = sb.tile([1, 64], f32, name="ones")
        nc.gpsimd.memset(ones[:, :], 1.0)

        # ---- resident tiles ----
        xq_sb = sb.tile([128, NDT, S], bf16, name="xq_sb")
        xk_sb = sb.tile([128, NDT, S], bf16, name="xk_sb")
        xv_sb = sb.tile([128, NDT, S], bf16, name="xv_sb")
        wq_sb = sb.tile([128, NDT, CPC], bf16, name="wq_sb")
        wk_sb = sb.tile([128, NDT, CPC], bf16, name="wk_sb")
        wv_sb = sb.tile([128, NDT, CPC], bf16, name="wv_sb")
        qT = [sb.tile([128, S], bf16, name=f"qT{p}") for p in range(NP)]
        kT = [sb.tile([128, S], bf16, name=f"kT{p}") for p in range(NP)]
        # V augmented with ones column: [sk_in_tile, sk_tile, head, 65]
        v_aug = sb.tile([128, NSK, 8, 65], bf16, name="v_aug")
        nc.gpsimd.memset(v_aug[:, :, :, 64:65], 1.0)

        def emit_proj_qk(p, w_sb, bias_sb, dst):
            """dst[p] = (x @ W + b).T rows 128p..; contract D, all 4 sq chunks."""
            for t in range(NJ):
                prt = ps.tile([128, 2, SQC], f32, tag="sc", bufs=3,
                              name=f"prj_{p}_{t}")
                pr = prt[:, 0, :]
                for dd in range(NDT):
                    nc.tensor.matmul(
                        pr,
                        w_sb[:, dd, p * 128:(p + 1) * 128],
                        (xq_sb if dst is qT else xk_sb)[:, dd, t * SQC:(t + 1) * SQC],
                        start=(dd == 0), stop=(dd == NDT - 1))
                nc.vector.tensor_scalar_add(
                    dst[p][:, t * SQC:(t + 1) * SQC], pr, bias_sb[:, p:p + 1])

        def emit_proj_v(tt_list):
            """v_aug[:, tt, h, 0:64] = (xv @ Wv + bv) rows tt*128.., bf16."""
            for tt in tt_list:
                pvt = ps.tile([128, 2, SQC], f32, tag="sc", bufs=3,
                              name=f"pv_{tt}")
                pv = pvt[:, 0, :]
                for dd in range(NDT):
                    nc.tensor.matmul(
                        pv,
                        xv_sb[:, dd, tt * 128:(tt + 1) * 128],
                        wv_sb[:, dd, :],
                        start=(dd == 0), stop=(dd == NDT - 1))
                nc.vector.tensor_add(
                    v_aug[:, tt, :, 0:64],
                    pv.rearrange("c (h d) -> c h d", h=8),
                    bv_bc.rearrange("c (h d) -> c h d", h=8))

        alpha = {}  # (p, j, h, g) -> tile [128, 2, 512] bf16 (sk pair g)

        def emit_scores_group(p, j, g):
            """QK^T + exp for sk-pair g of (pair p, sq chunk j)."""
            sc = []
            for h in range(2):
                s_h = ps.tile([128, 2, SQC], f32, tag="sc", bufs=3,
                              name=f"sc_{p}_{j}_{g}_{h}")
                sc.append(s_h)
            for u in range(2):
                i = 2 * g + u
                for h in range(2):
                    nc.tensor.matmul(
                        sc[h][:, u, :],
                        kT[p][h * 64:(h + 1) * 64, i * 128:(i + 1) * 128],
                        qT[p][h * 64:(h + 1) * 64, j * SQC:(j + 1) * SQC],
                        start=True, stop=True)
            for h in range(2):
                a_t = sb.tile([128, 2, SQC], bf16, tag="alpha", bufs=12,
                              name=f"al_{p}_{j}_{g}_{h}")
                nc.scalar.activation(a_t[:, :, :], sc[h][:, :, :], EXP, scale=0.125)
                alpha[(p, j, h, g)] = a_t

        av_live = {}  # (p, j, h) -> psum tile

        def emit_av_chunk(p, j, h, g):
            """Accumulate sk-pair g of alpha into av for (p,j,h)."""
            if (p, j, h) not in av_live:
                av_live[(p, j, h)] = ps.tile([65, SQC], f32, tag="av", bufs=2,
                                             name=f"av_{p}_{j}_{h}")
            av = av_live[(p, j, h)]
            a_t = alpha.pop((p, j, h, g))
            for u in range(2):
                i = 2 * g + u
                nc.tensor.matmul(
                    av[:, :],
                    v_aug[:, i, 2 * p + h, :],
                    a_t[:, u, :],
                    start=(i == 0), stop=(i == NSK - 1))

        def emit_norm_out(p, j):
            """Normalize both heads of (p,j) and DMA the [128,512] block out."""
            cx = sb.tile([128, SQC], bf16, tag="cx", bufs=2, name=f"cx_{p}_{j}")
            for h in range(2):
                av = av_live.pop((p, j, h))
                rec = sb.tile([1, SQC], f32r, tag="rec", bufs=2,
                              name=f"rec_{p}_{j}_{h}")
                nc.vector.reciprocal(rec[:, :], av[64:65, :])
                bct = ps.tile([128, 2, SQC], f32, tag="sc", bufs=3,
                              name=f"bc_{p}_{j}_{h}")
                nc.tensor.matmul(bct[0:64, 0, :], ones[:, :].bitcast(f32r), rec[:, :],
                                 start=True, stop=True)
                bcs = sb.tile([64, SQC], f32, tag="bcs", bufs=2,
                              name=f"bcs_{p}_{j}_{h}")
                nc.vector.tensor_copy(bcs[:, :], bct[0:64, 0, :])
                nc.vector.tensor_mul(
                    cx[h * 64:(h + 1) * 64, :], av[0:64, :], bcs[:, :])
            nc.sync.dma_start(
                out=outd[p * 128:(p + 1) * 128, j * SQC:(j + 1) * SQC],
                in_=cx[:, :])

        # ---- emission schedule (one rep) ----
        def _emit_all():
            # input DMAs; d-major tiles [128, dd, *]; x split by s-chunk so the
            # first projection tile only waits for 1/4 of the tensor
            nc.sync.dma_start(out=wk_sb, in_=wd[1].rearrange("(d p) c -> p d c", p=128))
            for t in range(NJ):
                nc.sync.dma_start(
                    out=xk_sb[:, :, t * SQC:(t + 1) * SQC],
                    in_=xd[1].rearrange("(d p) s -> p d s", p=128)[:, :, t * SQC:(t + 1) * SQC])
            nc.sync.dma_start(out=wq_sb, in_=wd[0].rearrange("(d p) c -> p d c", p=128))
            for t in range(NJ):
                nc.sync.dma_start(
                    out=xq_sb[:, :, t * SQC:(t + 1) * SQC],
                    in_=xd[0].rearrange("(d p) s -> p d s", p=128)[:, :, t * SQC:(t + 1) * SQC])
            nc.sync.dma_start(out=wv_sb, in_=wd[2].rearrange("(d p) c -> p d c", p=128))
            nc.sync.dma_start(out=xv_sb, in_=xd[2].rearrange("(d p) s -> p d s", p=128))

            emit_proj_qk(0, wk_sb, bk_sb, kT)
            emit_proj_qk(0, wq_sb, bq_sb, qT)

            seq = [(p, j) for p in range(NP) for j in range(NJ)]
            NG = NSK // 2
            LAG = 4  # av chunks trail scores by 4 sk-pair groups
            pending = None  # (p,j) whose norm+out is deferred one group
            for idx, (p, j) in enumerate(seq):
                for g in range(NG):
                    emit_scores_group(p, j, g)
                    if g == 1 and pending is not None:
                        emit_norm_out(*pending)
                        pending = None
                    if idx == 0:
                        # fill PE while ACT chews the first scores: V proj
                        emit_proj_v([2 * g, 2 * g + 1])
                    if g >= LAG:
                        emit_av_chunk(p, j, 0, g - LAG)
                        emit_av_chunk(p, j, 1, g - LAG)
                    # stage the next pair's projections one j-chunk early
                    if j == 2 and g in (1, 5) and p + 1 < NP:
                        emit_proj_qk(p + 1, wk_sb if g == 1 else wq_sb,
                                     bk_sb if g == 1 else bq_sb,
                                     kT if g == 1 else qT)
                for g in range(NG - LAG, NG):
                    emit_av_chunk(p, j, 0, g)
                    emit_av_chunk(p, j, 1, g)
                pending = (p, j)
            emit_norm_out(*pending)

        for _rep in range(reps):
            _emit_all()

    return nc


_NC_BY_REPS = {}


def _get_nc(reps=1):
    if reps not in _NC_BY_REPS:
        _install_drainfix()
        _NC_BY_REPS[reps] = _build_nc(reps)
    return _NC_BY_REPS[reps]


def _to_bf16(a):
    import ml_dtypes
    return np.asarray(a, np.float32).astype(ml_dtypes.bfloat16)


def make_in_maps(query, key_in, value, Wq, bq, Wk, bk, Wv, bv):
    query = np.asarray(query, np.float32)
    key_in = np.asarray(key_in, np.float32)
    value = np.asarray(value, np.float32)
    W3 = np.stack([np.asarray(Wq, np.float32), np.asarray(Wk, np.float32),
                   np.asarray(Wv, np.float32)])
    b3 = np.stack([np.asarray(bq, np.float32), np.asarray(bk, np.float32),
                   np.asarray(bv, np.float32)])
    in_maps = []
    for c in range(NCORES):
        b, hg = divmod(c, 2)
        cols = slice(hg * CPC, (hg + 1) * CPC)
        x3 = np.stack([query[b].T, key_in[b].T, value[b].T])
        in_maps.append({
            "x": np.ascontiguousarray(_to_bf16(x3)),
            "w": np.ascontiguousarray(_to_bf16(W3[:, :, cols])),
            "bias": np.ascontiguousarray(b3[:, cols]),
        })
    return in_maps


# ---------------------------------------------------------------- entry
def kernel(query, key_in, value, Wq, bq, Wk, bk, Wv, bv):
    from concourse.bass_utils import run_bass_kernel_spmd

    nc = _get_nc()
    in_maps = make_in_maps(query, key_in, value, Wq, bq, Wk, bk, Wv, bv)
    res = run_bass_kernel_spmd(nc, in_maps, core_ids=list(range(NCORES)))

    out = np.empty((B, S, D), np.float32)
    for c in range(NCORES):
        b, hg = divmod(c, 2)
        out[b, :, hg * CPC:(hg + 1) * CPC] = \
            np.asarray(res.results[c]["out"], np.float32).T
    return out
